# revision 1
# baseline (speedup 1.0000x reference)
"""Trainium2 Bass kernel for nn_DK_50414326120800 (dense_cnn, 8 cores).

Data-parallel over batch: 16 batches -> 2 per NeuronCore. Train-mode
BatchNorm statistics are exchanged with four tiny per-branch AllReduces
([128,4] fp32 each), scheduled branch-major so each collective overlaps the
other branch's compute.

Per-core pipeline (channels on partitions, 2 chunks of 128; pixels free dim;
bf16 data plane with fp32 PSUM/stats):
  per branch: DMA x (bf16) -> pool 16x16 (DVE reduce) -> conv_r (bf16
    TensorE matmul, fp32 PSUM) -> y1 bf16 via ACT evict with fused
    per-channel sums (accum_out) + sumsq (one scalar_tensor_tensor);
    ker-gen matmuls emitted AFTER conv_r to avoid PE head-of-line blocking
    on DVE pooling; per-branch AllReduce -> BN scale/shift (reciprocal +
    sqrt + Newton rsqrt).
  phase B per branch: BN+ReLU fused in one ACT pass into a zero-padded
    67x68 image; dynamic 4x4 grouped conv = 16 diagonal matmuls
    (diag_t = ident * ker[:,t] per-partition scalar) accumulating in PSUM
    over shifted APs; conv_b; y2 bf16 + stats; AllReduce; final BN+ReLU
    (split ACT/DVE) -> fp32 out -> DMA.

Conv biases are dropped (they cancel exactly under train-mode BN); pooling
1/256 mean factor is folded into the kernel-generator weights host-side.
"""

import sys
from contextlib import ExitStack

import numpy as np

sys.path.insert(0, "/opt/trn_rl_repo")

import ml_dtypes  # noqa: E402
import concourse.bacc as bacc  # noqa: E402
import concourse.mybir as mybir  # noqa: E402
import concourse.tile as tile  # noqa: E402
from concourse.bass_utils import run_bass_kernel_spmd  # noqa: E402

N_CORES = 8
B, CI, C, H, W = 16, 256, 256, 64, 64
BL = B // N_CORES            # local batches per core = 2
NK = 2                       # channel chunks of 128
PIX = H * W                  # 4096
FS = 4
EPS = 1e-5
NTOT = float(B * H * W)      # BN normalizer 65536
HP, WP = 67, 68              # padded image (top2/bot1, left2/right1+1 spare col)
F32 = mybir.dt.float32
BF16 = mybir.dt.bfloat16
AF = mybir.ActivationFunctionType
OP = mybir.AluOpType

_CACHE = {}

# (im, k) chunk-images whose dynamic-conv taps run on VectorE (bf16 STT)
DVE_IMG = set()
SPLIT_EVICTS = False  # alternate PSUM evictions between ACT and DVE
SPLIT_FINAL = False   # route half the final BN applies to DVE (2-pass)
BUFS = {"x": 2, "img": 9, "pad": 4, "guide": 3, "out": 2, "diag": 16}


def build(debug=False):
    nc = bacc.Bacc("TRN2", target_bir_lowering=False, num_devices=N_CORES)

    # ---- DRAM I/O --------------------------------------------------------
    xf_d = nc.dram_tensor("xf", [BL, NK, 128, PIX], BF16, kind="ExternalInput")
    xe_d = nc.dram_tensor("xe", [BL, NK, 128, PIX], BF16, kind="ExternalInput")
    w_in = {}
    for nm in ["wrf", "wre", "wbf", "wbe"]:
        for k in range(NK):
            w_in[f"{nm}T{k}"] = nc.dram_tensor(
                f"{nm}T{k}", [128, C], BF16, kind="ExternalInput")
    for nm in ["wkf", "wke"]:
        for k in range(NK):
            w_in[f"{nm}T{k}"] = nc.dram_tensor(
                f"{nm}T{k}", [128, C], F32, kind="ExternalInput")
    bkf_d = nc.dram_tensor("bkf", [128, 2], F32, kind="ExternalInput")
    bke_d = nc.dram_tensor("bke", [128, 2], F32, kind="ExternalInput")
    g1p_d = nc.dram_tensor("g1p", [128, 4], F32, kind="ExternalInput")
    be1p_d = nc.dram_tensor("be1p", [128, 4], F32, kind="ExternalInput")
    g2p_d = nc.dram_tensor("g2p", [128, 4], F32, kind="ExternalInput")
    be2p_d = nc.dram_tensor("be2p", [128, 4], F32, kind="ExternalInput")
    id_d = nc.dram_tensor("identbf", [128, 128], BF16, kind="ExternalInput")
    gf_d = nc.dram_tensor("gf", [BL, NK, 128, PIX], F32, kind="ExternalOutput")
    ge_d = nc.dram_tensor("ge", [BL, NK, 128, PIX], F32, kind="ExternalOutput")
    dbg = {}
    if debug:
        dbg["pooled"] = nc.dram_tensor("dbg_pooled", [BL, 2, NK, 128, 16], F32,
                                       kind="ExternalOutput")
        dbg["ker"] = nc.dram_tensor("dbg_ker", [BL, 2, NK, 128, 16], F32,
                                    kind="ExternalOutput")
        dbg["y1"] = nc.dram_tensor("dbg_y1", [4, NK, 128, PIX], BF16,
                                   kind="ExternalOutput")
        dbg["guide"] = nc.dram_tensor("dbg_guide", [4, NK, 128, PIX], BF16,
                                      kind="ExternalOutput")
        dbg["y2"] = nc.dram_tensor("dbg_y2", [4, NK, 128, PIX], BF16,
                                   kind="ExternalOutput")
        dbg["sc1"] = nc.dram_tensor("dbg_sc1", [128, 4], F32, kind="ExternalOutput")
        dbg["sh1"] = nc.dram_tensor("dbg_sh1", [128, 4], F32, kind="ExternalOutput")
        dbg["st1"] = nc.dram_tensor("dbg_st1", [128, 8], F32, kind="ExternalOutput")

    with tile.TileContext(nc) as tc, ExitStack() as ctx:
        cpool = ctx.enter_context(tc.tile_pool(name="consts", bufs=1))
        xpool = ctx.enter_context(tc.tile_pool(name="xin", bufs=BUFS["x"]))
        imgpool = ctx.enter_context(tc.tile_pool(name="img", bufs=BUFS["img"]))
        padpool = ctx.enter_context(tc.tile_pool(name="pads", bufs=BUFS["pad"]))
        gpool = ctx.enter_context(tc.tile_pool(name="guide", bufs=BUFS["guide"]))
        opool = ctx.enter_context(tc.tile_pool(name="outst", bufs=BUFS["out"]))
        dpool = ctx.enter_context(tc.tile_pool(name="diags", bufs=BUFS["diag"]))
        spool = ctx.enter_context(tc.tile_pool(name="small", bufs=1))
        pspool = ctx.enter_context(tc.tile_pool(name="ps", bufs=2, space="PSUM"))
        drpool = ctx.enter_context(tc.tile_pool(name="drb", bufs=1, space="DRAM"))

        # ---- first compute inputs in, then the rest of the constants ----
        x_first = {}
        for k in range(NK):
            t = xpool.tile([128, PIX], BF16, name=f"x_0_0_{k}", tag="x")
            nc.sync.dma_start(out=t[:, :], in_=xf_d[0, k])
            x_first[k] = t
        wt = {}
        for nm, dt_ in [("wrf", BF16), ("wre", BF16), ("wbf", BF16),
                        ("wbe", BF16), ("wkf", F32), ("wke", F32)]:
            for k in range(NK):
                t = cpool.tile([128, C], dt_, name=f"sb_{nm}T{k}", tag=f"sb_{nm}T{k}")
                nc.sync.dma_start(out=t[:, :], in_=w_in[f"{nm}T{k}"][:, :])
                wt[(nm, k)] = t
        bk_sb = {}
        for nm, d in [("bkf", bkf_d), ("bke", bke_d)]:
            t = cpool.tile([128, 2], F32, name=f"sb_{nm}", tag=f"sb_{nm}")
            nc.sync.dma_start(out=t[:, :], in_=d[:, :])
            bk_sb[nm] = t
        packs = {}
        for nm, d in [("g1p", g1p_d), ("be1p", be1p_d), ("g2p", g2p_d),
                      ("be2p", be2p_d)]:
            t = cpool.tile([128, 4], F32, name=f"sb_{nm}", tag=f"sb_{nm}")
            nc.sync.dma_start(out=t[:, :], in_=d[:, :])
            packs[nm] = t
        ident = cpool.tile([128, 128], BF16, name="sb_ident", tag="sb_ident")
        nc.sync.dma_start(out=ident[:, :], in_=id_d[:, :])

        # persistent small tiles
        scrA = spool.tile([128, 32], F32, name="scrA", tag="scrA")
        ssqA = spool.tile([128, 8], F32, name="ssqA", tag="ssqA")
        scrB = spool.tile([128, 32], F32, name="scrB", tag="scrB")
        ssqB = spool.tile([128, 8], F32, name="ssqB", tag="ssqB")
        pooled = {}
        for b in range(BL):
            for br in range(2):
                for k in range(NK):
                    pooled[(b, br, k)] = spool.tile(
                        [128, 16], F32, name=f"pool_{b}_{br}_{k}", tag="pooled",
                        bufs=BL * 2 * NK)
        kers = {}
        for b in range(BL):
            for br in range(2):
                for m in range(NK):
                    kers[(b, br, m)] = spool.tile(
                        [128, 16], F32, name=f"ker_{b}_{br}_{m}", tag="kers",
                        bufs=BL * 2 * NK)

        y1 = {}
        y2 = {}

        # ================= PHASE A =======================================
        # branch-major so each branch's BN1 AllReduce overlaps the other
        # branch's compute
        xdram = {0: xf_d, 1: xe_d}
        gst1_ = {}

        def do_pool(b, br, xt):
            for k in range(NK):
                s1 = spool.tile([128, 256], F32, name=f"s1_{b}_{br}_{k}",
                                tag="s1", bufs=2)
                x4 = xt[k].rearrange("p (y xb xi) -> p y xb xi", y=64, xb=4,
                                     xi=16)
                nc.vector.tensor_reduce(
                    out=s1.rearrange("p (y xb) -> p y xb", y=64, xb=4),
                    in_=x4, axis=mybir.AxisListType.X, op=OP.add)
                s2 = s1.rearrange("p (yb yi xb) -> p yb xb yi", yb=4,
                                  yi=16, xb=4)
                nc.vector.tensor_reduce(
                    out=pooled[(b, br, k)].rearrange(
                        "p (yb xb) -> p yb xb", yb=4, xb=4),
                    in_=s2, axis=mybir.AxisListType.X, op=OP.add)

        def do_kergen(b, br):
            knm = "wkf" if br == 0 else "wke"
            bnm = "bkf" if br == 0 else "bke"
            for m in range(NK):
                kps = pspool.tile([128, 1024], F32, name=f"kgp_{b}_{br}_{m}",
                                  tag="mmps", bufs=2)
                for k in range(NK):
                    nc.tensor.matmul(
                        kps[:, 0:16],
                        wt[(knm, k)][:, m * 128:(m + 1) * 128],
                        pooled[(b, br, k)][:, :],
                        start=(k == 0), stop=(k == NK - 1))
                nc.vector.tensor_scalar(
                    out=kers[(b, br, m)][:, :], in0=kps[:, 0:16],
                    scalar1=bk_sb[bnm][:, m:m + 1], scalar2=None,
                    op0=OP.add)

        for br in range(2):
            scr = spool.tile([128, 16], F32, name=f"scrA{br}", tag=f"scrA{br}")
            ssq = spool.tile([128, 4], F32, name=f"ssqA{br}", tag=f"ssqA{br}")
            for b in range(BL):
                if b == 0 and br == 0:
                    xt = x_first
                else:
                    xt = {}
                    for k in range(NK):
                        t = xpool.tile([128, PIX], BF16,
                                       name=f"x_{b}_{br}_{k}", tag="x")
                        nc.sync.dma_start(out=t[:, :], in_=xdram[br][b, k])
                        xt[k] = t
                do_pool(b, br, xt)

                # conv_r: y1[im, m] = sum_k wrT[k][:,m] @ x[k]
                rnm = "wrf" if br == 0 else "wre"
                im = b * 2 + br
                for m in range(NK):
                    yt = imgpool.tile([128, PIX], BF16, name=f"y1_{im}_{m}",
                                      tag="img")
                    y1[(im, m)] = yt
                    for q in range(4):
                        mp = pspool.tile([128, 1024], F32,
                                         name=f"rp_{im}_{m}_{q}", tag="mmps",
                                         bufs=2)
                        for n in range(2):
                            off = q * 1024 + n * 512
                            for k in range(NK):
                                nc.tensor.matmul(
                                    mp[:, n * 512:(n + 1) * 512],
                                    wt[(rnm, k)][:, m * 128:(m + 1) * 128],
                                    xt[k][:, off:off + 512],
                                    start=(k == 0), stop=(k == NK - 1))
                        g = (b * 2 + m) * 4 + q
                        nc.scalar.activation(
                            yt[:, q * 1024:(q + 1) * 1024], mp[:, :], AF.Copy,
                            accum_out=scr[:, g:g + 1])
                    # sumsq of this chunk-image
                    jk = opool.tile([128, PIX], BF16, name=f"jka_{im}_{m}",
                                    tag="outst")
                    nc.vector.scalar_tensor_tensor(
                        out=jk[:, :], in0=yt[:, :], scalar=1.0, in1=yt[:, :],
                        op0=OP.mult, op1=OP.mult,
                        accum_out=ssq[:, b * 2 + m:b * 2 + m + 1])
                    if debug:
                        nc.sync.dma_start(out=dbg["y1"][im, m], in_=yt[:, :])
                # ker-gen emitted after conv_r so the PE queue isn't
                # head-of-line blocked waiting on DVE pooling
                do_kergen(b, br)
            if debug and br == 1:
                for bb in range(BL):
                    for brr in range(2):
                        for k in range(NK):
                            nc.sync.dma_start(out=dbg["pooled"][bb, brr, k],
                                              in_=pooled[(bb, brr, k)][:, :])
                            nc.sync.dma_start(out=dbg["ker"][bb, brr, k],
                                              in_=kers[(bb, brr, k)][:, :])

            # ---- per-branch AR1 ----
            sums = spool.tile([128, 4], F32, name=f"sumsA{br}", tag=f"sumsA{br}")
            nc.vector.tensor_reduce(
                out=sums[:, :],
                in_=scr.rearrange("p (g q) -> p g q", g=4, q=4),
                axis=mybir.AxisListType.X, op=OP.add)
            loc = spool.tile([128, 4], F32, name=f"loc1{br}", tag=f"loc1{br}")
            nc.vector.tensor_tensor(out=loc[:, 0:2], in0=sums[:, 0:2],
                                    in1=sums[:, 2:4], op=OP.add)
            nc.vector.tensor_tensor(out=loc[:, 2:4], in0=ssq[:, 0:2],
                                    in1=ssq[:, 2:4], op=OP.add)
            cin = drpool.tile([128, 4], F32, name=f"cc1i{br}", tag=f"cc1i{br}")
            cout = drpool.tile([128, 4], F32, name=f"cc1o{br}", tag=f"cc1o{br}",
                               addr_space="Shared")
            nc.sync.dma_start(out=cin[:, :], in_=loc[:, :])
            nc.gpsimd.collective_compute(
                "AllReduce", OP.add, replica_groups=[list(range(N_CORES))],
                ins=[cin[:, :]], outs=[cout[:, :]])
            g1 = spool.tile([128, 4], F32, name=f"gst1{br}", tag=f"gst1{br}")
            nc.sync.dma_start(out=g1[:, :], in_=cout[:, :])
            gst1_[br] = g1

        def bn_coeffs(gst, gpack, bepack, pfx):
            """per-branch global (sum, sumsq) [128,4] -> scale/shift [128,2]."""
            mean = spool.tile([128, 2], F32, name=f"{pfx}_mean", tag=f"{pfx}_mean")
            nc.vector.tensor_scalar(out=mean[:, :], in0=gst[:, 0:2],
                                    scalar1=1.0 / NTOT, scalar2=None, op0=OP.mult)
            vpe = spool.tile([128, 2], F32, name=f"{pfx}_vpe", tag=f"{pfx}_vpe")
            nc.vector.tensor_scalar(out=vpe[:, :], in0=gst[:, 2:4],
                                    scalar1=1.0 / NTOT, scalar2=EPS,
                                    op0=OP.mult, op1=OP.add)
            msq = spool.tile([128, 2], F32, name=f"{pfx}_msq", tag=f"{pfx}_msq")
            nc.vector.tensor_tensor(out=msq[:, :], in0=mean[:, :],
                                    in1=mean[:, :], op=OP.mult)
            nc.vector.tensor_tensor(out=vpe[:, :], in0=vpe[:, :],
                                    in1=msq[:, :], op=OP.subtract)
            rcp = spool.tile([128, 2], F32, name=f"{pfx}_rcp", tag=f"{pfx}_rcp")
            nc.vector.reciprocal(rcp[:, :], vpe[:, :])
            r0 = spool.tile([128, 2], F32, name=f"{pfx}_r0", tag=f"{pfx}_r0")
            nc.scalar.activation(r0[:, :], rcp[:, :], AF.Sqrt)
            t1 = spool.tile([128, 2], F32, name=f"{pfx}_t1", tag=f"{pfx}_t1")
            nc.vector.tensor_tensor(out=t1[:, :], in0=r0[:, :], in1=r0[:, :],
                                    op=OP.mult)
            nc.vector.tensor_tensor(out=t1[:, :], in0=vpe[:, :], in1=t1[:, :],
                                    op=OP.mult)
            nc.vector.tensor_scalar(out=t1[:, :], in0=t1[:, :], scalar1=-0.5,
                                    scalar2=1.5, op0=OP.mult, op1=OP.add)
            nc.vector.tensor_tensor(out=r0[:, :], in0=r0[:, :], in1=t1[:, :],
                                    op=OP.mult)
            sc = spool.tile([128, 2], F32, name=f"{pfx}_sc", tag=f"{pfx}_sc")
            nc.vector.tensor_tensor(out=sc[:, :], in0=gpack[:, :],
                                    in1=r0[:, :], op=OP.mult)
            sh = spool.tile([128, 2], F32, name=f"{pfx}_sh", tag=f"{pfx}_sh")
            nc.vector.tensor_tensor(out=sh[:, :], in0=mean[:, :],
                                    in1=sc[:, :], op=OP.mult)
            nc.vector.tensor_tensor(out=sh[:, :], in0=bepack[:, :],
                                    in1=sh[:, :], op=OP.subtract)
            return sc, sh

        sc1 = {}
        sh1 = {}
        for br in range(2):
            sc1[br], sh1[br] = bn_coeffs(
                gst1_[br], packs["g1p"][:, br * 2:br * 2 + 2],
                packs["be1p"][:, br * 2:br * 2 + 2], f"c1{br}")

        # ================= PHASE B (branch-major) ========================
        gst2_ = {}
        for br in range(2):
            scr = spool.tile([128, 16], F32, name=f"scrB{br}", tag=f"scrB{br}")
            ssq = spool.tile([128, 4], F32, name=f"ssqB{br}", tag=f"ssqB{br}")
            for b in range(BL):
                im = b * 2 + br
                pads = {}
                for k in range(NK):
                    pt = padpool.tile([128, HP * WP], BF16,
                                      name=f"pad_{im}_{k}", tag="pad")
                    nc.gpsimd.memset(pt[:, :], 0.0)
                    p3 = pt.rearrange("p (h w) -> p h w", h=HP, w=WP)
                    nc.scalar.activation(
                        p3[:, 2:66, 2:66],
                        y1[(im, k)].rearrange("p (h w) -> p h w", h=64, w=64),
                        AF.Relu, bias=sh1[br][:, k:k + 1],
                        scale=sc1[br][:, k:k + 1])
                    pads[k] = p3

                # diagonal tap matrices from the OTHER branch's kernels
                diags = {}
                for k in range(NK):
                    if (im, k) in DVE_IMG:
                        continue
                    kt = kers[(b, 1 - br, k)]
                    for t in range(16):
                        dt_ = dpool.tile([128, 128], BF16,
                                         name=f"dg_{im}_{k}_{t}", tag="diag")
                        nc.vector.tensor_scalar(
                            out=dt_[:, :], in0=ident[:, :],
                            scalar1=kt[:, t:t + 1], scalar2=None, op0=OP.mult)
                        diags[(k, t)] = dt_
                guide = {}
                for k in range(NK):
                    gt = gpool.tile([128, PIX], BF16, name=f"gd_{im}_{k}",
                                    tag="guide")
                    guide[k] = gt
                    p3 = pads[k]
                    if (im, k) in DVE_IMG:
                        kt = kers[(b, 1 - br, k)]
                        g3 = gt.rearrange("p (h w) -> p h w", h=64, w=64)
                        nc.vector.tensor_scalar(
                            out=g3[:, :, :], in0=p3[:, 0:64, 0:64],
                            scalar1=kt[:, 0:1], scalar2=None, op0=OP.mult)
                        for t in range(1, 16):
                            i, j = t // 4, t % 4
                            nc.vector.scalar_tensor_tensor(
                                out=g3[:, :, :], in0=p3[:, i:i + 64, j:j + 64],
                                scalar=kt[:, t:t + 1], in1=g3[:, :, :],
                                op0=OP.mult, op1=OP.add)
                        continue
                    for q in range(4):
                        dp = pspool.tile([128, 1024], F32,
                                         name=f"dp_{im}_{k}_{q}", tag="dynps",
                                         bufs=2)
                        for t in range(16):
                            i, j = t // 4, t % 4
                            for n in range(2):
                                r0_ = q * 16 + n * 8 + i
                                nc.tensor.matmul(
                                    dp[:, n * 512:(n + 1) * 512],
                                    diags[(k, t)][:, :],
                                    p3[:, r0_:r0_ + 8, j:j + 64],
                                    start=(t == 0), stop=(t == 15))
                        nc.scalar.activation(
                            gt[:, q * 1024:(q + 1) * 1024], dp[:, :], AF.Copy)
                    if debug:
                        nc.sync.dma_start(out=dbg["guide"][im, k], in_=gt[:, :])

                # conv_b: y2[im, m] = sum_k wbT[k][:,m] @ guide[k]
                bnm2 = "wbf" if br == 0 else "wbe"
                for m in range(NK):
                    yt = imgpool.tile([128, PIX], BF16, name=f"y2_{im}_{m}",
                                      tag="img")
                    y2[(im, m)] = yt
                    for q in range(4):
                        mp = pspool.tile([128, 1024], F32,
                                         name=f"bp_{im}_{m}_{q}", tag="mmps",
                                         bufs=2)
                        for n in range(2):
                            off = q * 1024 + n * 512
                            for k in range(NK):
                                nc.tensor.matmul(
                                    mp[:, n * 512:(n + 1) * 512],
                                    wt[(bnm2, k)][:, m * 128:(m + 1) * 128],
                                    guide[k][:, off:off + 512],
                                    start=(k == 0), stop=(k == NK - 1))
                        g = (b * 2 + m) * 4 + q
                        nc.scalar.activation(
                            yt[:, q * 1024:(q + 1) * 1024], mp[:, :], AF.Copy,
                            accum_out=scr[:, g:g + 1])
                    jk = opool.tile([128, PIX], BF16, name=f"jkb_{im}_{m}",
                                    tag="outst")
                    nc.vector.scalar_tensor_tensor(
                        out=jk[:, :], in0=yt[:, :], scalar=1.0, in1=yt[:, :],
                        op0=OP.mult, op1=OP.mult,
                        accum_out=ssq[:, b * 2 + m:b * 2 + m + 1])
                    if debug:
                        nc.sync.dma_start(out=dbg["y2"][im, m], in_=yt[:, :])

            # ---- per-branch AR2 ----
            sums = spool.tile([128, 4], F32, name=f"sumsB{br}", tag=f"sumsB{br}")
            nc.vector.tensor_reduce(
                out=sums[:, :],
                in_=scr.rearrange("p (g q) -> p g q", g=4, q=4),
                axis=mybir.AxisListType.X, op=OP.add)
            loc = spool.tile([128, 4], F32, name=f"loc2{br}", tag=f"loc2{br}")
            nc.vector.tensor_tensor(out=loc[:, 0:2], in0=sums[:, 0:2],
                                    in1=sums[:, 2:4], op=OP.add)
            nc.vector.tensor_tensor(out=loc[:, 2:4], in0=ssq[:, 0:2],
                                    in1=ssq[:, 2:4], op=OP.add)
            cin = drpool.tile([128, 4], F32, name=f"cc2i{br}", tag=f"cc2i{br}")
            cout = drpool.tile([128, 4], F32, name=f"cc2o{br}", tag=f"cc2o{br}",
                               addr_space="Shared")
            nc.sync.dma_start(out=cin[:, :], in_=loc[:, :])
            nc.gpsimd.collective_compute(
                "AllReduce", OP.add, replica_groups=[list(range(N_CORES))],
                ins=[cin[:, :]], outs=[cout[:, :]])
            g2 = spool.tile([128, 4], F32, name=f"gst2{br}", tag=f"gst2{br}")
            nc.sync.dma_start(out=g2[:, :], in_=cout[:, :])
            gst2_[br] = g2

        sc2 = {}
        sh2 = {}
        for br in range(2):
            sc2[br], sh2[br] = bn_coeffs(
                gst2_[br], packs["g2p"][:, br * 2:br * 2 + 2],
                packs["be2p"][:, br * 2:br * 2 + 2], f"c2{br}")

        # ---- final BN+ReLU -> fp32 -> DMA out ---------------------------
        outdram = {0: gf_d, 1: ge_d}
        for br in range(2):
            for b in range(BL):
                im = b * 2 + br
                for m in range(NK):
                    # quartered so the post-AllReduce tail pipelines
                    # apply -> DMA instead of draining whole images
                    ot = opool.tile([128, PIX], F32, name=f"o_{im}_{m}",
                                    tag="outst")
                    for q in range(4):
                        sl = slice(q * 1024, (q + 1) * 1024)
                        if m == 0:
                            nc.vector.tensor_scalar(
                                out=ot[:, sl], in0=y2[(im, m)][:, sl],
                                scalar1=sc2[br][:, m:m + 1],
                                scalar2=sh2[br][:, m:m + 1],
                                op0=OP.mult, op1=OP.add)
                            nc.vector.tensor_scalar(
                                out=ot[:, sl], in0=ot[:, sl], scalar1=0.0,
                                scalar2=None, op0=OP.max)
                        else:
                            nc.scalar.activation(
                                ot[:, sl], y2[(im, m)][:, sl], AF.Relu,
                                bias=sh2[br][:, m:m + 1],
                                scale=sc2[br][:, m:m + 1])
                        nc.sync.dma_start(out=outdram[br][b, m][:, sl],
                                          in_=ot[:, sl])
        if debug:
            nc.sync.dma_start(out=dbg["sc1"][:, 0:2], in_=sc1[0][:, :])
            nc.sync.dma_start(out=dbg["sc1"][:, 2:4], in_=sc1[1][:, :])
            nc.sync.dma_start(out=dbg["sh1"][:, 0:2], in_=sh1[0][:, :])
            nc.sync.dma_start(out=dbg["sh1"][:, 2:4], in_=sh1[1][:, :])

    nc.compile()
    return nc


def _prep_maps(xf, xe, w_kf, b_kf, w_ke, b_ke, w_rf, g_rf, be_rf, w_re, g_re,
               be_re, w_bf, g_bf, be_bf, w_be, g_be, be_be):
    bf = ml_dtypes.bfloat16
    common = {}
    for nm, w, dt_ in [("wrf", w_rf, bf), ("wre", w_re, bf), ("wbf", w_bf, bf),
                       ("wbe", w_be, bf), ("wkf", w_kf / 256.0, np.float32),
                       ("wke", w_ke / 256.0, np.float32)]:
        wT = np.ascontiguousarray(np.asarray(w, np.float32).T.astype(dt_))
        for k in range(NK):
            common[f"{nm}T{k}"] = wT[k * 128:(k + 1) * 128]
    common["bkf"] = np.ascontiguousarray(
        np.asarray(b_kf, np.float32).reshape(2, 128).T)
    common["bke"] = np.ascontiguousarray(
        np.asarray(b_ke, np.float32).reshape(2, 128).T)

    def pack(gf_, ge_):
        p = np.zeros((128, 4), np.float32)
        for br in range(2):
            for m in range(NK):
                v = gf_ if br == 0 else ge_
                p[:, br * 2 + m] = np.asarray(v, np.float32)[
                    m * 128:(m + 1) * 128]
        return p

    common["g1p"] = pack(g_rf, g_re)
    common["be1p"] = pack(be_rf, be_re)
    common["g2p"] = pack(g_bf, g_be)
    common["be2p"] = pack(be_bf, be_be)
    common["identbf"] = np.eye(128, dtype=np.float32).astype(bf)

    xf = np.asarray(xf, np.float32).reshape(N_CORES, BL, NK, 128, PIX)
    xe = np.asarray(xe, np.float32).reshape(N_CORES, BL, NK, 128, PIX)
    maps = []
    for c in range(N_CORES):
        m = dict(common)
        m["xf"] = xf[c].astype(bf)
        m["xe"] = xe[c].astype(bf)
        maps.append(m)
    return maps


def kernel(xf, xe, w_kf, b_kf, w_ke, b_ke,
           w_rf, b_rf, g_rf, be_rf, w_re, b_re, g_re, be_re,
           w_bf, b_bf, g_bf, be_bf, w_be, b_be, g_be, be_be):
    # note: conv biases feeding a train-mode BatchNorm cancel exactly
    # (BN subtracts the batch mean), so b_rf/b_re/b_bf/b_be are unused.
    try:
        import jax
        jax.config.update("jax_compilation_cache_dir", "/tmp/jaxcache_kernel")
        jax.config.update("jax_persistent_cache_min_entry_size_bytes", 0)
        jax.config.update("jax_persistent_cache_min_compile_time_secs", 0)
    except Exception:
        pass
    if "nc" not in _CACHE:
        _CACHE["nc"] = build()
    nc = _CACHE["nc"]
    maps = _prep_maps(xf, xe, w_kf, b_kf, w_ke, b_ke, w_rf, g_rf, be_rf,
                      w_re, g_re, be_re, w_bf, g_bf, be_bf, w_be, g_be, be_be)
    res = run_bass_kernel_spmd(nc, maps, core_ids=list(range(N_CORES)))
    gf = np.concatenate([r["gf"].reshape(BL, C, H, W) for r in res.results])
    ge = np.concatenate([r["ge"].reshape(BL, C, H, W) for r in res.results])
    return gf.astype(np.float32), ge.astype(np.float32)



# revision 4
# speedup vs baseline: 1.0639x; 1.0639x over previous
"""Trainium2 Bass kernel for nn_DK_50414326120800 (dense_cnn, 8 cores).

Data-parallel over batch: 16 batches -> 2 per NeuronCore. Train-mode
BatchNorm statistics are exchanged with four tiny per-branch collectives,
implemented as AllGather + local 3-step pairwise reduce (the cost of an
AllGather is ~15us vs ~28us for AllReduce: fixed overhead x1.875).

Per-core pipeline (channels on partitions, 2 chunks of 128; pixels free dim;
bf16 data plane with fp32 PSUM/stats):
  per branch: DMA x (bf16) -> pool 16x16 (DVE reduce) -> conv_r (bf16
    TensorE matmul, fp32 PSUM) -> y1 bf16 via ACT evict with fused
    per-channel sums (accum_out) + sumsq (one scalar_tensor_tensor);
    ker-gen matmuls emitted AFTER conv_r to avoid PE head-of-line blocking
    on DVE pooling; per-branch AllGather -> BN scale/shift (reciprocal +
    sqrt + Newton rsqrt).
  phase B per branch: BN+ReLU fused in one ACT pass into a zero-bordered
    67x68 image (border-only memsets); dynamic 4x4 grouped conv = 16
    diagonal matmuls (diag_t = ident * ker[:,t] per-partition scalar)
    accumulating in PSUM over shifted APs; conv_b; y2 bf16 + stats;
    AllGather; final BN+ReLU -> bf16 out -> DMA (host casts to fp32).
  The br0 final applies are emitted mid-br1 (own buffer tag so the
  scheduler can hoist them into br1's compute window); br1's tail applies
  are split ACT/DVE to shorten the post-collective tail.

Conv biases are dropped (they cancel exactly under train-mode BN); pooling
1/256 mean factor is folded into the kernel-generator weights host-side.
"""

import sys
from contextlib import ExitStack

import numpy as np

sys.path.insert(0, "/opt/trn_rl_repo")

import ml_dtypes  # noqa: E402
import concourse.bacc as bacc  # noqa: E402
import concourse.mybir as mybir  # noqa: E402
import concourse.tile as tile  # noqa: E402
from concourse.bass_utils import run_bass_kernel_spmd  # noqa: E402

N_CORES = 8
B, CI, C, H, W = 16, 256, 256, 64, 64
BL = B // N_CORES            # local batches per core = 2
NK = 2                       # channel chunks of 128
PIX = H * W                  # 4096
FS = 4
EPS = 1e-5
NTOT = float(B * H * W)      # BN normalizer 65536
HP, WP = 67, 68              # padded image (top2/bot1, left2/right1+1 spare col)
F32 = mybir.dt.float32
BF16 = mybir.dt.bfloat16
AF = mybir.ActivationFunctionType
OP = mybir.AluOpType

_CACHE = {}

BUFS = {"x": 2, "img": 9, "pad": 4, "guide": 3, "out": 3, "scr": 2,
        "diag": 16}


def build(debug=False):
    nc = bacc.Bacc("TRN2", target_bir_lowering=False, num_devices=N_CORES)

    # ---- DRAM I/O --------------------------------------------------------
    xf_d = nc.dram_tensor("xf", [BL, NK, 128, PIX], BF16, kind="ExternalInput")
    xe_d = nc.dram_tensor("xe", [BL, NK, 128, PIX], BF16, kind="ExternalInput")
    w_in = {}
    for nm in ["wrf", "wre", "wbf", "wbe"]:
        for k in range(NK):
            w_in[f"{nm}T{k}"] = nc.dram_tensor(
                f"{nm}T{k}", [128, C], BF16, kind="ExternalInput")
    for nm in ["wkf", "wke"]:
        for k in range(NK):
            w_in[f"{nm}T{k}"] = nc.dram_tensor(
                f"{nm}T{k}", [128, C], F32, kind="ExternalInput")
    bkf_d = nc.dram_tensor("bkf", [128, 2], F32, kind="ExternalInput")
    bke_d = nc.dram_tensor("bke", [128, 2], F32, kind="ExternalInput")
    g1p_d = nc.dram_tensor("g1p", [128, 4], F32, kind="ExternalInput")
    be1p_d = nc.dram_tensor("be1p", [128, 4], F32, kind="ExternalInput")
    g2p_d = nc.dram_tensor("g2p", [128, 4], F32, kind="ExternalInput")
    be2p_d = nc.dram_tensor("be2p", [128, 4], F32, kind="ExternalInput")
    id_d = nc.dram_tensor("identbf", [128, 128], BF16, kind="ExternalInput")
    gf_d = nc.dram_tensor("gf", [BL, NK, 128, PIX], BF16,
                          kind="ExternalOutput")
    ge_d = nc.dram_tensor("ge", [BL, NK, 128, PIX], BF16,
                          kind="ExternalOutput")

    with tile.TileContext(nc) as tc, ExitStack() as ctx:
        cpool = ctx.enter_context(tc.tile_pool(name="consts", bufs=1))
        xpool = ctx.enter_context(tc.tile_pool(name="xin", bufs=BUFS["x"]))
        imgpool = ctx.enter_context(tc.tile_pool(name="img", bufs=BUFS["img"]))
        padpool = ctx.enter_context(tc.tile_pool(name="pads", bufs=BUFS["pad"]))
        gpool = ctx.enter_context(tc.tile_pool(name="guide", bufs=BUFS["guide"]))
        opool = ctx.enter_context(tc.tile_pool(name="outst", bufs=BUFS["out"]))
        scrpool = ctx.enter_context(tc.tile_pool(name="scrp", bufs=BUFS["scr"]))
        dpool = ctx.enter_context(tc.tile_pool(name="diags", bufs=BUFS["diag"]))
        spool = ctx.enter_context(tc.tile_pool(name="small", bufs=1))
        pspool = ctx.enter_context(tc.tile_pool(name="ps", bufs=2, space="PSUM"))
        drpool = ctx.enter_context(tc.tile_pool(name="drb", bufs=1, space="DRAM"))

        # ---- first compute inputs in, then the rest of the constants ----
        x_first = {}
        for k in range(NK):
            t = xpool.tile([128, PIX], BF16, name=f"x_0_0_{k}", tag="x")
            nc.sync.dma_start(out=t[:, :], in_=xf_d[0, k])
            x_first[k] = t
        wt = {}
        for nm, dt_ in [("wrf", BF16), ("wre", BF16), ("wbf", BF16),
                        ("wbe", BF16), ("wkf", F32), ("wke", F32)]:
            for k in range(NK):
                t = cpool.tile([128, C], dt_, name=f"sb_{nm}T{k}", tag=f"sb_{nm}T{k}")
                nc.sync.dma_start(out=t[:, :], in_=w_in[f"{nm}T{k}"][:, :])
                wt[(nm, k)] = t
        bk_sb = {}
        for nm, d in [("bkf", bkf_d), ("bke", bke_d)]:
            t = cpool.tile([128, 2], F32, name=f"sb_{nm}", tag=f"sb_{nm}")
            nc.sync.dma_start(out=t[:, :], in_=d[:, :])
            bk_sb[nm] = t
        packs = {}
        for nm, d in [("g1p", g1p_d), ("be1p", be1p_d), ("g2p", g2p_d),
                      ("be2p", be2p_d)]:
            t = cpool.tile([128, 4], F32, name=f"sb_{nm}", tag=f"sb_{nm}")
            nc.sync.dma_start(out=t[:, :], in_=d[:, :])
            packs[nm] = t
        ident = cpool.tile([128, 128], BF16, name="sb_ident", tag="sb_ident")
        nc.sync.dma_start(out=ident[:, :], in_=id_d[:, :])

        pooled = {}
        for b in range(BL):
            for br in range(2):
                for k in range(NK):
                    pooled[(b, br, k)] = spool.tile(
                        [128, 16], F32, name=f"pool_{b}_{br}_{k}", tag="pooled",
                        bufs=BL * 2 * NK)
        kers = {}
        for b in range(BL):
            for br in range(2):
                for m in range(NK):
                    kers[(b, br, m)] = spool.tile(
                        [128, 16], F32, name=f"ker_{b}_{br}_{m}", tag="kers",
                        bufs=BL * 2 * NK)

        y1 = {}
        y2 = {}

        # ---- collective: AllGather + local pairwise reduce --------------
        def ag_allreduce(pfx, loc):
            cin = drpool.tile([128, 4], F32, name=f"{pfx}i", tag=f"{pfx}i")
            cout = drpool.tile([N_CORES, 128, 4], F32, name=f"{pfx}o",
                               tag=f"{pfx}o", addr_space="Shared")
            nc.sync.dma_start(out=cin[:, :], in_=loc[:, :])
            nc.gpsimd.collective_compute(
                "AllGather", OP.bypass,
                replica_groups=[list(range(N_CORES))],
                ins=[cin[:, :]], outs=[cout[:, :, :]])
            gth = spool.tile([128, 32], F32, name=f"{pfx}g", tag=f"{pfx}g")
            nc.sync.dma_start(
                out=gth.rearrange("p (n f) -> p n f", n=N_CORES, f=4),
                in_=cout.rearrange("n p f -> p n f"))
            t16 = spool.tile([128, 16], F32, name=f"{pfx}h", tag=f"{pfx}h")
            nc.vector.tensor_tensor(out=t16[:, :], in0=gth[:, 0:16],
                                    in1=gth[:, 16:32], op=OP.add)
            t8 = spool.tile([128, 8], F32, name=f"{pfx}q", tag=f"{pfx}q")
            nc.vector.tensor_tensor(out=t8[:, :], in0=t16[:, 0:8],
                                    in1=t16[:, 8:16], op=OP.add)
            g = spool.tile([128, 4], F32, name=f"{pfx}r", tag=f"{pfx}r")
            nc.vector.tensor_tensor(out=g[:, :], in0=t8[:, 0:4],
                                    in1=t8[:, 4:8], op=OP.add)
            return g

        # ================= PHASE A =======================================
        # branch-major so each branch's BN1 collective overlaps the other
        # branch's compute
        xdram = {0: xf_d, 1: xe_d}
        gst1_ = {}

        def do_pool(b, br, xt):
            for k in range(NK):
                s1 = spool.tile([128, 256], F32, name=f"s1_{b}_{br}_{k}",
                                tag="s1", bufs=2)
                x4 = xt[k].rearrange("p (y xb xi) -> p y xb xi", y=64, xb=4,
                                     xi=16)
                nc.vector.tensor_reduce(
                    out=s1.rearrange("p (y xb) -> p y xb", y=64, xb=4),
                    in_=x4, axis=mybir.AxisListType.X, op=OP.add)
                s2 = s1.rearrange("p (yb yi xb) -> p yb xb yi", yb=4,
                                  yi=16, xb=4)
                nc.vector.tensor_reduce(
                    out=pooled[(b, br, k)].rearrange(
                        "p (yb xb) -> p yb xb", yb=4, xb=4),
                    in_=s2, axis=mybir.AxisListType.X, op=OP.add)

        def do_kergen(b, br):
            knm = "wkf" if br == 0 else "wke"
            bnm = "bkf" if br == 0 else "bke"
            for m in range(NK):
                kps = pspool.tile([128, 1024], F32, name=f"kgp_{b}_{br}_{m}",
                                  tag="mmps", bufs=2)
                for k in range(NK):
                    nc.tensor.matmul(
                        kps[:, 0:16],
                        wt[(knm, k)][:, m * 128:(m + 1) * 128],
                        pooled[(b, br, k)][:, :],
                        start=(k == 0), stop=(k == NK - 1))
                nc.vector.tensor_scalar(
                    out=kers[(b, br, m)][:, :], in0=kps[:, 0:16],
                    scalar1=bk_sb[bnm][:, m:m + 1], scalar2=None,
                    op0=OP.add)

        for br in range(2):
            scr = spool.tile([128, 16], F32, name=f"scrA{br}", tag=f"scrA{br}")
            ssq = spool.tile([128, 4], F32, name=f"ssqA{br}", tag=f"ssqA{br}")
            for b in range(BL):
                if b == 0 and br == 0:
                    xt = x_first
                else:
                    xt = {}
                    for k in range(NK):
                        t = xpool.tile([128, PIX], BF16,
                                       name=f"x_{b}_{br}_{k}", tag="x")
                        nc.sync.dma_start(out=t[:, :], in_=xdram[br][b, k])
                        xt[k] = t
                do_pool(b, br, xt)

                # conv_r: y1[im, m] = sum_k wrT[k][:,m] @ x[k]
                rnm = "wrf" if br == 0 else "wre"
                im = b * 2 + br
                for m in range(NK):
                    yt = imgpool.tile([128, PIX], BF16, name=f"y1_{im}_{m}",
                                      tag="img")
                    y1[(im, m)] = yt
                    for q in range(4):
                        mp = pspool.tile([128, 1024], F32,
                                         name=f"rp_{im}_{m}_{q}", tag="mmps",
                                         bufs=2)
                        for n in range(2):
                            off = q * 1024 + n * 512
                            for k in range(NK):
                                nc.tensor.matmul(
                                    mp[:, n * 512:(n + 1) * 512],
                                    wt[(rnm, k)][:, m * 128:(m + 1) * 128],
                                    xt[k][:, off:off + 512],
                                    start=(k == 0), stop=(k == NK - 1))
                        g = (b * 2 + m) * 4 + q
                        nc.scalar.activation(
                            yt[:, q * 1024:(q + 1) * 1024], mp[:, :], AF.Copy,
                            accum_out=scr[:, g:g + 1])
                    # sumsq of this chunk-image
                    jk = scrpool.tile([128, PIX], BF16, name=f"jka_{im}_{m}",
                                      tag="scr")
                    nc.vector.scalar_tensor_tensor(
                        out=jk[:, :], in0=yt[:, :], scalar=1.0, in1=yt[:, :],
                        op0=OP.mult, op1=OP.mult,
                        accum_out=ssq[:, b * 2 + m:b * 2 + m + 1])
                # ker-gen emitted after conv_r so the PE queue isn't
                # head-of-line blocked waiting on DVE pooling
                do_kergen(b, br)

            # ---- per-branch stats collective ----
            sums = spool.tile([128, 4], F32, name=f"sumsA{br}", tag=f"sumsA{br}")
            nc.vector.tensor_reduce(
                out=sums[:, :],
                in_=scr.rearrange("p (g q) -> p g q", g=4, q=4),
                axis=mybir.AxisListType.X, op=OP.add)
            loc = spool.tile([128, 4], F32, name=f"loc1{br}", tag=f"loc1{br}")
            nc.vector.tensor_tensor(out=loc[:, 0:2], in0=sums[:, 0:2],
                                    in1=sums[:, 2:4], op=OP.add)
            nc.vector.tensor_tensor(out=loc[:, 2:4], in0=ssq[:, 0:2],
                                    in1=ssq[:, 2:4], op=OP.add)
            gst1_[br] = ag_allreduce(f"cc1{br}", loc)

        def bn_coeffs(gst, gpack, bepack, pfx):
            """per-branch global (sum, sumsq) [128,4] -> scale/shift [128,2]."""
            mean = spool.tile([128, 2], F32, name=f"{pfx}_mean", tag=f"{pfx}_mean")
            nc.vector.tensor_scalar(out=mean[:, :], in0=gst[:, 0:2],
                                    scalar1=1.0 / NTOT, scalar2=None, op0=OP.mult)
            vpe = spool.tile([128, 2], F32, name=f"{pfx}_vpe", tag=f"{pfx}_vpe")
            nc.vector.tensor_scalar(out=vpe[:, :], in0=gst[:, 2:4],
                                    scalar1=1.0 / NTOT, scalar2=EPS,
                                    op0=OP.mult, op1=OP.add)
            msq = spool.tile([128, 2], F32, name=f"{pfx}_msq", tag=f"{pfx}_msq")
            nc.vector.tensor_tensor(out=msq[:, :], in0=mean[:, :],
                                    in1=mean[:, :], op=OP.mult)
            nc.vector.tensor_tensor(out=vpe[:, :], in0=vpe[:, :],
                                    in1=msq[:, :], op=OP.subtract)
            rcp = spool.tile([128, 2], F32, name=f"{pfx}_rcp", tag=f"{pfx}_rcp")
            nc.vector.reciprocal(rcp[:, :], vpe[:, :])
            r0 = spool.tile([128, 2], F32, name=f"{pfx}_r0", tag=f"{pfx}_r0")
            nc.scalar.activation(r0[:, :], rcp[:, :], AF.Sqrt)
            t1 = spool.tile([128, 2], F32, name=f"{pfx}_t1", tag=f"{pfx}_t1")
            nc.vector.tensor_tensor(out=t1[:, :], in0=r0[:, :], in1=r0[:, :],
                                    op=OP.mult)
            nc.vector.tensor_tensor(out=t1[:, :], in0=vpe[:, :], in1=t1[:, :],
                                    op=OP.mult)
            nc.vector.tensor_scalar(out=t1[:, :], in0=t1[:, :], scalar1=-0.5,
                                    scalar2=1.5, op0=OP.mult, op1=OP.add)
            nc.vector.tensor_tensor(out=r0[:, :], in0=r0[:, :], in1=t1[:, :],
                                    op=OP.mult)
            sc = spool.tile([128, 2], F32, name=f"{pfx}_sc", tag=f"{pfx}_sc")
            nc.vector.tensor_tensor(out=sc[:, :], in0=gpack[:, :],
                                    in1=r0[:, :], op=OP.mult)
            sh = spool.tile([128, 2], F32, name=f"{pfx}_sh", tag=f"{pfx}_sh")
            nc.vector.tensor_tensor(out=sh[:, :], in0=mean[:, :],
                                    in1=sc[:, :], op=OP.mult)
            nc.vector.tensor_tensor(out=sh[:, :], in0=bepack[:, :],
                                    in1=sh[:, :], op=OP.subtract)
            return sc, sh

        sc1 = {}
        sh1 = {}
        for br in range(2):
            sc1[br], sh1[br] = bn_coeffs(
                gst1_[br], packs["g1p"][:, br * 2:br * 2 + 2],
                packs["be1p"][:, br * 2:br * 2 + 2], f"c1{br}")

        # ================= PHASE B (branch-major) ========================
        outdram = {0: gf_d, 1: ge_d}
        sc2 = {}
        sh2 = {}

        def make_pads(b, br):
            """zero-bordered BN1+ReLU images for (b, br); border-only memset."""
            im = b * 2 + br
            pads = {}
            for k in range(NK):
                pt = padpool.tile([128, HP * WP], BF16,
                                  name=f"pad_{im}_{k}", tag="pad")
                p3 = pt.rearrange("p (h w) -> p h w", h=HP, w=WP)
                nc.gpsimd.memset(p3[:, 0:2, :], 0.0)      # top 2 rows
                nc.gpsimd.memset(p3[:, 66:67, :], 0.0)    # bottom row
                nc.gpsimd.memset(p3[:, 2:66, 0:2], 0.0)   # left 2 cols
                nc.gpsimd.memset(p3[:, 2:66, 66:68], 0.0)  # right 2 cols
                nc.scalar.activation(
                    p3[:, 2:66, 2:66],
                    y1[(im, k)].rearrange("p (h w) -> p h w", h=64, w=64),
                    AF.Relu, bias=sh1[br][:, k:k + 1],
                    scale=sc1[br][:, k:k + 1])
                pads[k] = p3
            return pads

        def make_diags(b, br):
            """diagonal tap matrices from the OTHER branch's kernels."""
            im = b * 2 + br
            diags = {}
            for k in range(NK):
                kt = kers[(b, 1 - br, k)]
                for t in range(16):
                    dt_ = dpool.tile([128, 128], BF16,
                                     name=f"dg_{im}_{k}_{t}", tag="diag")
                    nc.vector.tensor_scalar(
                        out=dt_[:, :], in0=ident[:, :],
                        scalar1=kt[:, t:t + 1], scalar2=None, op0=OP.mult)
                    diags[(k, t)] = dt_
            return diags

        def dyn_and_convb(b, br, pads, diags, scr, ssq):
            """dynamic conv + conv_b + y2 stats for image (b, br)."""
            im = b * 2 + br
            guide = {}
            for k in range(NK):
                gt = gpool.tile([128, PIX], BF16, name=f"gd_{im}_{k}",
                                tag="guide")
                guide[k] = gt
                p3 = pads[k]
                for q in range(4):
                    dp = pspool.tile([128, 1024], F32,
                                     name=f"dp_{im}_{k}_{q}", tag="dynps",
                                     bufs=2)
                    for t in range(16):
                        i, j = t // 4, t % 4
                        for n in range(2):
                            r0_ = q * 16 + n * 8 + i
                            nc.tensor.matmul(
                                dp[:, n * 512:(n + 1) * 512],
                                diags[(k, t)][:, :],
                                p3[:, r0_:r0_ + 8, j:j + 64],
                                start=(t == 0), stop=(t == 15))
                    nc.scalar.activation(
                        gt[:, q * 1024:(q + 1) * 1024], dp[:, :], AF.Copy)

            # conv_b: y2[im, m] = sum_k wbT[k][:,m] @ guide[k]
            bnm2 = "wbf" if br == 0 else "wbe"
            for m in range(NK):
                yt = imgpool.tile([128, PIX], BF16, name=f"y2_{im}_{m}",
                                  tag="img")
                y2[(im, m)] = yt
                for q in range(4):
                    mp = pspool.tile([128, 1024], F32,
                                     name=f"bp_{im}_{m}_{q}", tag="mmps",
                                     bufs=2)
                    for n in range(2):
                        off = q * 1024 + n * 512
                        for k in range(NK):
                            nc.tensor.matmul(
                                mp[:, n * 512:(n + 1) * 512],
                                wt[(bnm2, k)][:, m * 128:(m + 1) * 128],
                                guide[k][:, off:off + 512],
                                start=(k == 0), stop=(k == NK - 1))
                    g = (b * 2 + m) * 4 + q
                    nc.scalar.activation(
                        yt[:, q * 1024:(q + 1) * 1024], mp[:, :], AF.Copy,
                        accum_out=scr[:, g:g + 1])
                jk = scrpool.tile([128, PIX], BF16, name=f"jkb_{im}_{m}",
                                  tag="scr")
                nc.vector.scalar_tensor_tensor(
                    out=jk[:, :], in0=yt[:, :], scalar=1.0, in1=yt[:, :],
                    op0=OP.mult, op1=OP.mult,
                    accum_out=ssq[:, b * 2 + m:b * 2 + m + 1])

        def stats2(br, scr, ssq):
            sums = spool.tile([128, 4], F32, name=f"sumsB{br}", tag=f"sumsB{br}")
            nc.vector.tensor_reduce(
                out=sums[:, :],
                in_=scr.rearrange("p (g q) -> p g q", g=4, q=4),
                axis=mybir.AxisListType.X, op=OP.add)
            loc = spool.tile([128, 4], F32, name=f"loc2{br}", tag=f"loc2{br}")
            nc.vector.tensor_tensor(out=loc[:, 0:2], in0=sums[:, 0:2],
                                    in1=sums[:, 2:4], op=OP.add)
            nc.vector.tensor_tensor(out=loc[:, 2:4], in0=ssq[:, 0:2],
                                    in1=ssq[:, 2:4], op=OP.add)
            return ag_allreduce(f"cc2{br}", loc)

        def final_apply(b, br, m, engine):
            """BN2+ReLU -> bf16 -> DMA out, quartered for DMA pipelining."""
            im = b * 2 + br
            ot = opool.tile([128, PIX], BF16, name=f"o_{im}_{m}", tag="outb")
            for q in range(4):
                sl = slice(q * 1024, (q + 1) * 1024)
                if engine == "dve":
                    nc.vector.tensor_scalar(
                        out=ot[:, sl], in0=y2[(im, m)][:, sl],
                        scalar1=sc2[br][:, m:m + 1],
                        scalar2=sh2[br][:, m:m + 1],
                        op0=OP.mult, op1=OP.add)
                    nc.vector.tensor_scalar(
                        out=ot[:, sl], in0=ot[:, sl], scalar1=0.0,
                        scalar2=None, op0=OP.max)
                else:
                    nc.scalar.activation(
                        ot[:, sl], y2[(im, m)][:, sl], AF.Relu,
                        bias=sh2[br][:, m:m + 1],
                        scale=sc2[br][:, m:m + 1])
                nc.sync.dma_start(out=outdram[br][b, m][:, sl],
                                  in_=ot[:, sl])

        # ---- br0 phase B ----
        scrB0 = spool.tile([128, 16], F32, name="scrB0", tag="scrB0")
        ssqB0 = spool.tile([128, 4], F32, name="ssqB0", tag="ssqB0")
        for b in range(BL):
            pads = make_pads(b, 0)
            diags = make_diags(b, 0)
            dyn_and_convb(b, 0, pads, diags, scrB0, ssqB0)
        gst2_0 = stats2(0, scrB0, ssqB0)

        # ---- br1 phase B, with br0 applies emitted mid-stream ----
        scrB1 = spool.tile([128, 16], F32, name="scrB1", tag="scrB1")
        ssqB1 = spool.tile([128, 4], F32, name="ssqB1", tag="ssqB1")
        pads = make_pads(0, 1)
        diags = make_diags(0, 1)
        dyn_and_convb(0, 1, pads, diags, scrB1, ssqB1)

        # im3's pads/diags before the br0 applies so they aren't queued
        # behind the collective wait
        pads3 = make_pads(1, 1)
        diags3 = make_diags(1, 1)

        sc2[0], sh2[0] = bn_coeffs(
            gst2_0, packs["g2p"][:, 0:2], packs["be2p"][:, 0:2], "c20")
        for b in range(BL):
            for m in range(NK):
                final_apply(b, 0, m, "dve" if m == 0 else "act")

        dyn_and_convb(1, 1, pads3, diags3, scrB1, ssqB1)
        gst2_1 = stats2(1, scrB1, ssqB1)
        sc2[1], sh2[1] = bn_coeffs(
            gst2_1, packs["g2p"][:, 2:4], packs["be2p"][:, 2:4], "c21")
        # tail applies split across engines
        for b in range(BL):
            final_apply(b, 1, 0, "dve")
            final_apply(b, 1, 1, "act")

    nc.compile()
    return nc


def _prep_maps(xf, xe, w_kf, b_kf, w_ke, b_ke, w_rf, g_rf, be_rf, w_re, g_re,
               be_re, w_bf, g_bf, be_bf, w_be, g_be, be_be):
    bf = ml_dtypes.bfloat16
    common = {}
    for nm, w, dt_ in [("wrf", w_rf, bf), ("wre", w_re, bf), ("wbf", w_bf, bf),
                       ("wbe", w_be, bf), ("wkf", w_kf / 256.0, np.float32),
                       ("wke", w_ke / 256.0, np.float32)]:
        wT = np.ascontiguousarray(np.asarray(w, np.float32).T.astype(dt_))
        for k in range(NK):
            common[f"{nm}T{k}"] = wT[k * 128:(k + 1) * 128]
    common["bkf"] = np.ascontiguousarray(
        np.asarray(b_kf, np.float32).reshape(2, 128).T)
    common["bke"] = np.ascontiguousarray(
        np.asarray(b_ke, np.float32).reshape(2, 128).T)

    def pack(gf_, ge_):
        p = np.zeros((128, 4), np.float32)
        for br in range(2):
            for m in range(NK):
                v = gf_ if br == 0 else ge_
                p[:, br * 2 + m] = np.asarray(v, np.float32)[
                    m * 128:(m + 1) * 128]
        return p

    common["g1p"] = pack(g_rf, g_re)
    common["be1p"] = pack(be_rf, be_re)
    common["g2p"] = pack(g_bf, g_be)
    common["be2p"] = pack(be_bf, be_be)
    common["identbf"] = np.eye(128, dtype=np.float32).astype(bf)

    xf = np.asarray(xf, np.float32).reshape(N_CORES, BL, NK, 128, PIX)
    xe = np.asarray(xe, np.float32).reshape(N_CORES, BL, NK, 128, PIX)
    maps = []
    for c in range(N_CORES):
        m = dict(common)
        m["xf"] = xf[c].astype(bf)
        m["xe"] = xe[c].astype(bf)
        maps.append(m)
    return maps


def kernel(xf, xe, w_kf, b_kf, w_ke, b_ke,
           w_rf, b_rf, g_rf, be_rf, w_re, b_re, g_re, be_re,
           w_bf, b_bf, g_bf, be_bf, w_be, b_be, g_be, be_be):
    # note: conv biases feeding a train-mode BatchNorm cancel exactly
    # (BN subtracts the batch mean), so b_rf/b_re/b_bf/b_be are unused.
    try:
        import jax
        jax.config.update("jax_compilation_cache_dir", "/tmp/jaxcache_kernel")
        jax.config.update("jax_persistent_cache_min_entry_size_bytes", 0)
        jax.config.update("jax_persistent_cache_min_compile_time_secs", 0)
    except Exception:
        pass
    if "nc" not in _CACHE:
        _CACHE["nc"] = build()
    nc = _CACHE["nc"]
    maps = _prep_maps(xf, xe, w_kf, b_kf, w_ke, b_ke, w_rf, g_rf, be_rf,
                      w_re, g_re, be_re, w_bf, g_bf, be_bf, w_be, g_be, be_be)
    res = run_bass_kernel_spmd(nc, maps, core_ids=list(range(N_CORES)))
    gf = np.concatenate(
        [np.asarray(r["gf"]).astype(np.float32).reshape(BL, C, H, W)
         for r in res.results])
    ge = np.concatenate(
        [np.asarray(r["ge"]).astype(np.float32).reshape(BL, C, H, W)
         for r in res.results])
    return gf, ge


# revision 17
# speedup vs baseline: 1.1028x; 1.0365x over previous
"""Trainium2 Bass kernel for nn_DK_50414326120800 (dense_cnn, 8 cores).

Data-parallel over batch: 16 batches -> 2 per NeuronCore. Train-mode
BatchNorm statistics are exchanged with four tiny per-branch collectives,
implemented as AllGather + local 3-step pairwise reduce (the cost of an
AllGather is ~15us vs ~28us for AllReduce: fixed overhead x1.875).

Per-core pipeline (channels on partitions, 2 chunks of 128; pixels free dim;
bf16 data plane with fp32 PSUM/stats):
  per branch: DMA x (bf16) -> pool 16x16 (DVE reduce) -> conv_r (bf16
    TensorE matmul, fp32 PSUM) -> y1 bf16 via ACT evict with fused
    per-channel sums (accum_out) + sumsq (one scalar_tensor_tensor);
    ker-gen matmuls emitted AFTER conv_r to avoid PE head-of-line blocking
    on DVE pooling; per-branch AllGather -> BN scale/shift (reciprocal +
    sqrt + Newton rsqrt).
  phase B per branch: BN+ReLU fused in one ACT pass into a zero-bordered
    67x68 image (border-only memsets); dynamic 4x4 grouped conv = 16
    diagonal matmuls (diag_t = ident * ker[:,t] per-partition scalar)
    accumulating in PSUM over shifted APs; conv_b; y2 bf16 + stats;
    AllGather; final BN+ReLU -> bf16 out -> DMA (host casts to fp32).
  The br0 final applies are emitted mid-br1 (own buffer tag so the
  scheduler can hoist them into br1's compute window); br1's tail applies
  are split ACT/DVE to shorten the post-collective tail.

Conv biases are dropped (they cancel exactly under train-mode BN); pooling
1/256 mean factor is folded into the kernel-generator weights host-side.
"""

import sys
from contextlib import ExitStack

import numpy as np

sys.path.insert(0, "/opt/trn_rl_repo")

import ml_dtypes  # noqa: E402
import concourse.bacc as bacc  # noqa: E402
import concourse.mybir as mybir  # noqa: E402
import concourse.tile as tile  # noqa: E402
from concourse.bass_utils import run_bass_kernel_spmd  # noqa: E402

N_CORES = 8
B, CI, C, H, W = 16, 256, 256, 64, 64
BL = B // N_CORES            # local batches per core = 2
NK = 2                       # channel chunks of 128
PIX = H * W                  # 4096
FS = 4
EPS = 1e-5
NTOT = float(B * H * W)      # BN normalizer 65536
HP, WP = 67, 68              # padded image (top2/bot1, left2/right1+1 spare col)
F32 = mybir.dt.float32
BF16 = mybir.dt.bfloat16
AF = mybir.ActivationFunctionType
OP = mybir.AluOpType

_CACHE = {}

BUFS = {"x": 3, "img": 9, "pad": 4, "guide": 3, "out": 3, "scr": 2,
        "diag": 16}


def build(debug=False):
    nc = bacc.Bacc("TRN2", target_bir_lowering=False, num_devices=N_CORES)

    # ---- DRAM I/O --------------------------------------------------------
    xf_d = nc.dram_tensor("xf", [BL, NK, 128, PIX], BF16, kind="ExternalInput")
    xe_d = nc.dram_tensor("xe", [BL, NK, 128, PIX], BF16, kind="ExternalInput")
    w_in = {}
    for nm in ["wrf", "wre", "wbf", "wbe"]:
        for k in range(NK):
            w_in[f"{nm}T{k}"] = nc.dram_tensor(
                f"{nm}T{k}", [128, C], BF16, kind="ExternalInput")
    for nm in ["wkf", "wke"]:
        for k in range(NK):
            w_in[f"{nm}T{k}"] = nc.dram_tensor(
                f"{nm}T{k}", [128, C], F32, kind="ExternalInput")
    bkf_d = nc.dram_tensor("bkf", [128, 2], F32, kind="ExternalInput")
    bke_d = nc.dram_tensor("bke", [128, 2], F32, kind="ExternalInput")
    g1p_d = nc.dram_tensor("g1p", [128, 4], F32, kind="ExternalInput")
    be1p_d = nc.dram_tensor("be1p", [128, 4], F32, kind="ExternalInput")
    g2p_d = nc.dram_tensor("g2p", [128, 4], F32, kind="ExternalInput")
    be2p_d = nc.dram_tensor("be2p", [128, 4], F32, kind="ExternalInput")
    id_d = nc.dram_tensor("identbf", [128, 128], BF16, kind="ExternalInput")
    gf_d = nc.dram_tensor("gf", [BL, NK, 128, PIX], BF16,
                          kind="ExternalOutput")
    ge_d = nc.dram_tensor("ge", [BL, NK, 128, PIX], BF16,
                          kind="ExternalOutput")

    with tile.TileContext(nc) as tc, ExitStack() as ctx:
        cpool = ctx.enter_context(tc.tile_pool(name="consts", bufs=1))
        xpool = ctx.enter_context(tc.tile_pool(name="xin", bufs=BUFS["x"]))
        imgpool = ctx.enter_context(tc.tile_pool(name="img", bufs=BUFS["img"]))
        padpool = ctx.enter_context(tc.tile_pool(name="pads", bufs=BUFS["pad"]))
        gpool = ctx.enter_context(tc.tile_pool(name="guide", bufs=BUFS["guide"]))
        opool = ctx.enter_context(tc.tile_pool(name="outst", bufs=BUFS["out"]))
        scrpool = ctx.enter_context(tc.tile_pool(name="scrp", bufs=BUFS["scr"]))
        dpool = ctx.enter_context(tc.tile_pool(name="diags", bufs=BUFS["diag"]))
        spool = ctx.enter_context(tc.tile_pool(name="small", bufs=1))
        pspool = ctx.enter_context(tc.tile_pool(name="ps", bufs=2, space="PSUM"))
        drpool = ctx.enter_context(tc.tile_pool(name="drb", bufs=1, space="DRAM"))

        # ---- conv_r weights first (first matmul needs them), then the
        # first image's x interleaved k-within-quarter, then the rest ----
        wt = {}

        def load_w(nm, dt_):
            for k in range(NK):
                t = cpool.tile([128, C], dt_, name=f"sb_{nm}T{k}",
                               tag=f"sb_{nm}T{k}")
                nc.sync.dma_start(out=t[:, :], in_=w_in[f"{nm}T{k}"][:, :])
                wt[(nm, k)] = t

        load_w("wrf", BF16)
        x_first = {}
        for k in range(NK):
            x_first[k] = xpool.tile([128, PIX], BF16, name=f"x_0_0_{k}",
                                    tag="x")
        for s in range(4):
            sl = slice(s * 1024, (s + 1) * 1024)
            for k in range(NK):
                nc.sync.dma_start(out=x_first[k][:, sl], in_=xf_d[0, k][:, sl])
        for nm, dt_ in [("wre", BF16), ("wbf", BF16), ("wbe", BF16),
                        ("wkf", F32), ("wke", F32)]:
            load_w(nm, dt_)
        bk_sb = {}
        for nm, d in [("bkf", bkf_d), ("bke", bke_d)]:
            t = cpool.tile([128, 2], F32, name=f"sb_{nm}", tag=f"sb_{nm}")
            nc.sync.dma_start(out=t[:, :], in_=d[:, :])
            bk_sb[nm] = t
        packs = {}
        for nm, d in [("g1p", g1p_d), ("be1p", be1p_d), ("g2p", g2p_d),
                      ("be2p", be2p_d)]:
            t = cpool.tile([128, 4], F32, name=f"sb_{nm}", tag=f"sb_{nm}")
            nc.sync.dma_start(out=t[:, :], in_=d[:, :])
            packs[nm] = t
        ident = cpool.tile([128, 128], BF16, name="sb_ident", tag="sb_ident")
        nc.sync.dma_start(out=ident[:, :], in_=id_d[:, :])

        pooled = {}
        for b in range(BL):
            for br in range(2):
                for k in range(NK):
                    pooled[(b, br, k)] = spool.tile(
                        [128, 16], F32, name=f"pool_{b}_{br}_{k}", tag="pooled",
                        bufs=BL * 2 * NK)
        kers = {}
        for b in range(BL):
            for br in range(2):
                for m in range(NK):
                    kers[(b, br, m)] = spool.tile(
                        [128, 16], F32, name=f"ker_{b}_{br}_{m}", tag="kers",
                        bufs=BL * 2 * NK)

        y1 = {}
        y2 = {}

        # ---- collective: AllGather + local pairwise reduce --------------
        def ag_allreduce(pfx, loc):
            cin = drpool.tile([128, 4], F32, name=f"{pfx}i", tag=f"{pfx}i")
            cout = drpool.tile([N_CORES, 128, 4], F32, name=f"{pfx}o",
                               tag=f"{pfx}o", addr_space="Shared")
            nc.sync.dma_start(out=cin[:, :], in_=loc[:, :])
            nc.gpsimd.collective_compute(
                "AllGather", OP.bypass,
                replica_groups=[list(range(N_CORES))],
                ins=[cin[:, :]], outs=[cout[:, :, :]])
            gth = spool.tile([128, 32], F32, name=f"{pfx}g", tag=f"{pfx}g")
            nc.sync.dma_start(
                out=gth.rearrange("p (n f) -> p n f", n=N_CORES, f=4),
                in_=cout.rearrange("n p f -> p n f"))
            t16 = spool.tile([128, 16], F32, name=f"{pfx}h", tag=f"{pfx}h")
            nc.gpsimd.tensor_tensor(out=t16[:, :], in0=gth[:, 0:16],
                                    in1=gth[:, 16:32], op=OP.add)
            t8 = spool.tile([128, 8], F32, name=f"{pfx}q", tag=f"{pfx}q")
            nc.gpsimd.tensor_tensor(out=t8[:, :], in0=t16[:, 0:8],
                                    in1=t16[:, 8:16], op=OP.add)
            g = spool.tile([128, 4], F32, name=f"{pfx}r", tag=f"{pfx}r")
            nc.gpsimd.tensor_tensor(out=g[:, :], in0=t8[:, 0:4],
                                    in1=t8[:, 4:8], op=OP.add)
            return g

        # ================= PHASE A =======================================
        # branch-major so each branch's BN1 collective overlaps the other
        # branch's compute
        xdram = {0: xf_d, 1: xe_d}
        gst1_ = {}

        def do_pool(b, br, xt):
            for k in range(NK):
                s1 = spool.tile([128, 256], F32, name=f"s1_{b}_{br}_{k}",
                                tag="s1", bufs=2)
                x4 = xt[k].rearrange("p (y xb xi) -> p y xb xi", y=64, xb=4,
                                     xi=16)
                nc.vector.tensor_reduce(
                    out=s1.rearrange("p (y xb) -> p y xb", y=64, xb=4),
                    in_=x4, axis=mybir.AxisListType.X, op=OP.add)
                s2 = s1.rearrange("p (yb yi xb) -> p yb xb yi", yb=4,
                                  yi=16, xb=4)
                nc.vector.tensor_reduce(
                    out=pooled[(b, br, k)].rearrange(
                        "p (yb xb) -> p yb xb", yb=4, xb=4),
                    in_=s2, axis=mybir.AxisListType.X, op=OP.add)

        def do_kergen(b, br):
            knm = "wkf" if br == 0 else "wke"
            bnm = "bkf" if br == 0 else "bke"
            for m in range(NK):
                kps = pspool.tile([128, 1024], F32, name=f"kgp_{b}_{br}_{m}",
                                  tag="mmps", bufs=2)
                for k in range(NK):
                    nc.tensor.matmul(
                        kps[:, 0:16],
                        wt[(knm, k)][:, m * 128:(m + 1) * 128],
                        pooled[(b, br, k)][:, :],
                        start=(k == 0), stop=(k == NK - 1))
                nc.vector.tensor_scalar(
                    out=kers[(b, br, m)][:, :], in0=kps[:, 0:16],
                    scalar1=bk_sb[bnm][:, m:m + 1], scalar2=None,
                    op0=OP.add)

        def reduce16_to4(t16, dst, pfx):
            """[128, (q=4, g=4)] q-major slots -> [128, 4] via two
            contiguous pairwise adds on Pool (strided gpsimd APs hang)."""
            s8 = spool.tile([128, 8], F32, name=f"s8{pfx}", tag=f"s8{pfx}")
            nc.gpsimd.tensor_tensor(out=s8[:, :], in0=t16[:, 0:8],
                                    in1=t16[:, 8:16], op=OP.add)
            nc.gpsimd.tensor_tensor(out=dst, in0=s8[:, 0:4],
                                    in1=s8[:, 4:8], op=OP.add)

        def reduce_stats(scr, ssq, pfx):
            """[128,16] quarter-slot (sums, sumsqs) -> [128,4] loc on Pool."""
            sums = spool.tile([128, 4], F32, name=f"sums{pfx}", tag=f"sums{pfx}")
            reduce16_to4(scr, sums[:, :], f"a{pfx}")
            sq4 = spool.tile([128, 4], F32, name=f"sq4{pfx}", tag=f"sq4{pfx}")
            reduce16_to4(ssq, sq4[:, :], f"b{pfx}")
            loc = spool.tile([128, 4], F32, name=f"loc{pfx}", tag=f"loc{pfx}")
            nc.gpsimd.tensor_tensor(out=loc[:, 0:2], in0=sums[:, 0:2],
                                    in1=sums[:, 2:4], op=OP.add)
            nc.gpsimd.tensor_tensor(out=loc[:, 2:4], in0=sq4[:, 0:2],
                                    in1=sq4[:, 2:4], op=OP.add)
            return loc

        for br in range(2):
            scr = spool.tile([128, 16], F32, name=f"scrA{br}", tag=f"scrA{br}")
            ssq = spool.tile([128, 16], F32, name=f"ssqA{br}", tag=f"ssqA{br}")
            # x DMAs for the whole branch upfront, then pooling for both
            # batches (so kergen never waits on DVE mid-branch)
            xts = {}
            for b in range(BL):
                if b == 0 and br == 0:
                    xts[b] = x_first
                    continue
                xt = {}
                for k in range(NK):
                    xt[k] = xpool.tile([128, PIX], BF16,
                                       name=f"x_{b}_{br}_{k}", tag="x")
                for s in range(4):
                    sl = slice(s * 1024, (s + 1) * 1024)
                    for k in range(NK):
                        nc.sync.dma_start(out=xt[k][:, sl],
                                          in_=xdram[br][b, k][:, sl])
                xts[b] = xt
            for b in range(BL):
                do_pool(b, br, xts[b])
            for b in range(BL):
                xt = xts[b]
                # conv_r: y1[im, m] = sum_k wrT[k][:,m] @ x[k]
                rnm = "wrf" if br == 0 else "wre"
                im = b * 2 + br
                for m in range(NK):
                    yt = imgpool.tile([128, PIX], BF16, name=f"y1_{im}_{m}",
                                      tag="img")
                    y1[(im, m)] = yt
                    for q in range(4):
                        mp = pspool.tile([128, 1024], F32,
                                         name=f"rp_{im}_{m}_{q}", tag="mmps",
                                         bufs=2)
                        for n in range(2):
                            off = q * 1024 + n * 512
                            for k in range(NK):
                                nc.tensor.matmul(
                                    mp[:, n * 512:(n + 1) * 512],
                                    wt[(rnm, k)][:, m * 128:(m + 1) * 128],
                                    xt[k][:, off:off + 512],
                                    start=(k == 0), stop=(k == NK - 1))
                        g = q * 4 + b * 2 + m
                        sl = slice(q * 1024, (q + 1) * 1024)
                        nc.scalar.activation(
                            yt[:, sl], mp[:, :], AF.Copy,
                            accum_out=scr[:, g:g + 1])
                        # per-quarter sumsq so stats trail the evict closely
                        jk = scrpool.tile([128, 1024], BF16,
                                          name=f"jka_{im}_{m}_{q}", tag="scr")
                        nc.vector.scalar_tensor_tensor(
                            out=jk[:, :], in0=yt[:, sl], scalar=1.0,
                            in1=yt[:, sl], op0=OP.mult, op1=OP.mult,
                            accum_out=ssq[:, g:g + 1])
                # ker-gen emitted after conv_r so the PE queue isn't
                # head-of-line blocked waiting on DVE pooling
                do_kergen(b, br)

            # ---- per-branch stats collective ----
            gst1_[br] = ag_allreduce(f"cc1{br}", reduce_stats(scr, ssq, f"A{br}"))

        def bn_coeffs(gst, gpack, bepack, pfx, eng="pool"):
            """per-branch global (sum, sumsq) [128,4] -> scale/shift [128,2].

            eng="pool" keeps the chain off DVE (mid-kernel, DVE busy);
            eng="dve" pipelines back-to-back ops with lower per-op latency
            (tail, DVE idle). Newton step refines the table-based rsqrt."""
            ve = nc.gpsimd if eng == "pool" else nc.vector
            mean = spool.tile([128, 2], F32, name=f"{pfx}_mean", tag=f"{pfx}_mean")
            ve.tensor_scalar(out=mean[:, :], in0=gst[:, 0:2],
                                    scalar1=1.0 / NTOT, scalar2=None, op0=OP.mult)
            vpe = spool.tile([128, 2], F32, name=f"{pfx}_vpe", tag=f"{pfx}_vpe")
            ve.tensor_scalar(out=vpe[:, :], in0=gst[:, 2:4],
                                    scalar1=1.0 / NTOT, scalar2=EPS,
                                    op0=OP.mult, op1=OP.add)
            msq = spool.tile([128, 2], F32, name=f"{pfx}_msq", tag=f"{pfx}_msq")
            ve.tensor_tensor(out=msq[:, :], in0=mean[:, :],
                             in1=mean[:, :], op=OP.mult)
            ve.tensor_tensor(out=vpe[:, :], in0=vpe[:, :],
                             in1=msq[:, :], op=OP.subtract)
            rcp = spool.tile([128, 2], F32, name=f"{pfx}_rcp", tag=f"{pfx}_rcp")
            nc.vector.reciprocal(rcp[:, :], vpe[:, :])
            r0 = spool.tile([128, 2], F32, name=f"{pfx}_r0", tag=f"{pfx}_r0")
            nc.scalar.activation(r0[:, :], rcp[:, :], AF.Sqrt)
            t1 = spool.tile([128, 2], F32, name=f"{pfx}_t1", tag=f"{pfx}_t1")
            ve.tensor_tensor(out=t1[:, :], in0=r0[:, :], in1=r0[:, :],
                             op=OP.mult)
            ve.tensor_tensor(out=t1[:, :], in0=vpe[:, :], in1=t1[:, :],
                             op=OP.mult)
            ve.tensor_scalar(out=t1[:, :], in0=t1[:, :], scalar1=-0.5,
                             scalar2=1.5, op0=OP.mult, op1=OP.add)
            ve.tensor_tensor(out=r0[:, :], in0=r0[:, :], in1=t1[:, :],
                             op=OP.mult)
            sc = spool.tile([128, 2], F32, name=f"{pfx}_sc", tag=f"{pfx}_sc")
            ve.tensor_tensor(out=sc[:, :], in0=gpack[:, :],
                             in1=r0[:, :], op=OP.mult)
            sh = spool.tile([128, 2], F32, name=f"{pfx}_sh", tag=f"{pfx}_sh")
            ve.tensor_tensor(out=sh[:, :], in0=mean[:, :],
                             in1=sc[:, :], op=OP.mult)
            ve.tensor_tensor(out=sh[:, :], in0=bepack[:, :],
                             in1=sh[:, :], op=OP.subtract)
            return sc, sh

        sc1 = {}
        sh1 = {}
        for br in range(2):
            sc1[br], sh1[br] = bn_coeffs(
                gst1_[br], packs["g1p"][:, br * 2:br * 2 + 2],
                packs["be1p"][:, br * 2:br * 2 + 2], f"c1{br}")

        # ================= PHASE B (branch-major) ========================
        outdram = {0: gf_d, 1: ge_d}
        sc2 = {}
        sh2 = {}

        def make_pads(b, br):
            """zero-bordered BN1+ReLU images for (b, br); border-only memset."""
            im = b * 2 + br
            pads = {}
            for k in range(NK):
                pt = padpool.tile([128, HP * WP], BF16,
                                  name=f"pad_{im}_{k}", tag="pad")
                p3 = pt.rearrange("p (h w) -> p h w", h=HP, w=WP)
                nc.gpsimd.memset(p3[:, 0:2, :], 0.0)      # top 2 rows
                nc.gpsimd.memset(p3[:, 66:67, :], 0.0)    # bottom row
                nc.gpsimd.memset(p3[:, 2:66, 0:2], 0.0)   # left 2 cols
                nc.gpsimd.memset(p3[:, 2:66, 66:68], 0.0)  # right 2 cols
                nc.scalar.activation(
                    p3[:, 2:66, 2:66],
                    y1[(im, k)].rearrange("p (h w) -> p h w", h=64, w=64),
                    AF.Relu, bias=sh1[br][:, k:k + 1],
                    scale=sc1[br][:, k:k + 1])
                pads[k] = p3
            return pads

        def make_diags(b, br):
            """diagonal tap matrices from the OTHER branch's kernels."""
            im = b * 2 + br
            diags = {}
            for k in range(NK):
                kt = kers[(b, 1 - br, k)]
                for t in range(16):
                    dt_ = dpool.tile([128, 128], BF16,
                                     name=f"dg_{im}_{k}_{t}", tag="diag")
                    nc.vector.tensor_scalar(
                        out=dt_[:, :], in0=ident[:, :],
                        scalar1=kt[:, t:t + 1], scalar2=None, op0=OP.mult)
                    diags[(k, t)] = dt_
            return diags

        def dyn_and_convb(b, br, pads, diags, scr, ssq):
            """dynamic conv + conv_b + y2 stats for image (b, br)."""
            im = b * 2 + br
            guide = {}
            for k in range(NK):
                gt = gpool.tile([128, PIX], BF16, name=f"gd_{im}_{k}",
                                tag="guide")
                guide[k] = gt
                p3 = pads[k]
                for q in range(4):
                    dp = pspool.tile([128, 1024], F32,
                                     name=f"dp_{im}_{k}_{q}", tag="dynps",
                                     bufs=2)
                    for t in range(16):
                        i, j = t // 4, t % 4
                        for n in range(2):
                            r0_ = q * 16 + n * 8 + i
                            nc.tensor.matmul(
                                dp[:, n * 512:(n + 1) * 512],
                                diags[(k, t)][:, :],
                                p3[:, r0_:r0_ + 8, j:j + 64],
                                start=(t == 0), stop=(t == 15))
                    nc.scalar.activation(
                        gt[:, q * 1024:(q + 1) * 1024], dp[:, :], AF.Copy)

            # conv_b: y2[im, m] = sum_k wbT[k][:,m] @ guide[k]
            bnm2 = "wbf" if br == 0 else "wbe"
            for m in range(NK):
                yt = imgpool.tile([128, PIX], BF16, name=f"y2_{im}_{m}",
                                  tag="img")
                y2[(im, m)] = yt
                for q in range(4):
                    mp = pspool.tile([128, 1024], F32,
                                     name=f"bp_{im}_{m}_{q}", tag="mmps",
                                     bufs=2)
                    for n in range(2):
                        off = q * 1024 + n * 512
                        for k in range(NK):
                            nc.tensor.matmul(
                                mp[:, n * 512:(n + 1) * 512],
                                wt[(bnm2, k)][:, m * 128:(m + 1) * 128],
                                guide[k][:, off:off + 512],
                                start=(k == 0), stop=(k == NK - 1))
                    g = q * 4 + b * 2 + m
                    sl = slice(q * 1024, (q + 1) * 1024)
                    nc.scalar.activation(
                        yt[:, sl], mp[:, :], AF.Copy,
                        accum_out=scr[:, g:g + 1])
                    jk = scrpool.tile([128, 1024], BF16,
                                      name=f"jkb_{im}_{m}_{q}", tag="scr")
                    nc.vector.scalar_tensor_tensor(
                        out=jk[:, :], in0=yt[:, sl], scalar=1.0,
                        in1=yt[:, sl], op0=OP.mult, op1=OP.mult,
                        accum_out=ssq[:, g:g + 1])

        def stats2(br, scr, ssq):
            return ag_allreduce(f"cc2{br}", reduce_stats(scr, ssq, f"B{br}"))

        def final_apply(b, br, m, engine):
            """BN2+ReLU -> bf16 -> DMA out, quartered for DMA pipelining."""
            im = b * 2 + br
            ot = opool.tile([128, PIX], BF16, name=f"o_{im}_{m}", tag="outb")
            for q in range(4):
                sl = slice(q * 1024, (q + 1) * 1024)
                if engine == "dve":
                    nc.vector.tensor_scalar(
                        out=ot[:, sl], in0=y2[(im, m)][:, sl],
                        scalar1=sc2[br][:, m:m + 1],
                        scalar2=sh2[br][:, m:m + 1],
                        op0=OP.mult, op1=OP.add)
                    nc.vector.tensor_scalar(
                        out=ot[:, sl], in0=ot[:, sl], scalar1=0.0,
                        scalar2=None, op0=OP.max)
                else:
                    nc.scalar.activation(
                        ot[:, sl], y2[(im, m)][:, sl], AF.Relu,
                        bias=sh2[br][:, m:m + 1],
                        scale=sc2[br][:, m:m + 1])
                nc.sync.dma_start(out=outdram[br][b, m][:, sl],
                                  in_=ot[:, sl])

        # ---- br0 phase B ----
        scrB0 = spool.tile([128, 16], F32, name="scrB0", tag="scrB0")
        ssqB0 = spool.tile([128, 16], F32, name="ssqB0", tag="ssqB0")
        for b in range(BL):
            pads = make_pads(b, 0)
            diags = make_diags(b, 0)
            dyn_and_convb(b, 0, pads, diags, scrB0, ssqB0)
        gst2_0 = stats2(0, scrB0, ssqB0)

        # ---- br1 phase B, with br0 applies emitted mid-stream ----
        scrB1 = spool.tile([128, 16], F32, name="scrB1", tag="scrB1")
        ssqB1 = spool.tile([128, 16], F32, name="ssqB1", tag="ssqB1")
        pads = make_pads(0, 1)
        diags = make_diags(0, 1)
        dyn_and_convb(0, 1, pads, diags, scrB1, ssqB1)

        # im3's pads/diags before the br0 applies so they aren't queued
        # behind the collective wait
        pads3 = make_pads(1, 1)
        diags3 = make_diags(1, 1)

        sc2[0], sh2[0] = bn_coeffs(
            gst2_0, packs["g2p"][:, 0:2], packs["be2p"][:, 0:2], "c20")
        for b in range(BL):
            for m in range(NK):
                final_apply(b, 0, m, "dve" if m == 0 else "act")

        dyn_and_convb(1, 1, pads3, diags3, scrB1, ssqB1)
        gst2_1 = stats2(1, scrB1, ssqB1)
        sc2[1], sh2[1] = bn_coeffs(
            gst2_1, packs["g2p"][:, 2:4], packs["be2p"][:, 2:4], "c21",
            eng="dve")
        # tail applies split across engines
        for b in range(BL):
            final_apply(b, 1, 0, "dve")
            final_apply(b, 1, 1, "act")

    nc.compile()
    return nc


def _prep_maps(xf, xe, w_kf, b_kf, w_ke, b_ke, w_rf, g_rf, be_rf, w_re, g_re,
               be_re, w_bf, g_bf, be_bf, w_be, g_be, be_be):
    bf = ml_dtypes.bfloat16
    common = {}
    for nm, w, dt_ in [("wrf", w_rf, bf), ("wre", w_re, bf), ("wbf", w_bf, bf),
                       ("wbe", w_be, bf), ("wkf", w_kf / 256.0, np.float32),
                       ("wke", w_ke / 256.0, np.float32)]:
        wT = np.ascontiguousarray(np.asarray(w, np.float32).T.astype(dt_))
        for k in range(NK):
            common[f"{nm}T{k}"] = wT[k * 128:(k + 1) * 128]
    common["bkf"] = np.ascontiguousarray(
        np.asarray(b_kf, np.float32).reshape(2, 128).T)
    common["bke"] = np.ascontiguousarray(
        np.asarray(b_ke, np.float32).reshape(2, 128).T)

    def pack(gf_, ge_):
        p = np.zeros((128, 4), np.float32)
        for br in range(2):
            for m in range(NK):
                v = gf_ if br == 0 else ge_
                p[:, br * 2 + m] = np.asarray(v, np.float32)[
                    m * 128:(m + 1) * 128]
        return p

    common["g1p"] = pack(g_rf, g_re)
    common["be1p"] = pack(be_rf, be_re)
    common["g2p"] = pack(g_bf, g_be)
    common["be2p"] = pack(be_bf, be_be)
    common["identbf"] = np.eye(128, dtype=np.float32).astype(bf)

    xf = np.asarray(xf, np.float32).reshape(N_CORES, BL, NK, 128, PIX)
    xe = np.asarray(xe, np.float32).reshape(N_CORES, BL, NK, 128, PIX)
    maps = []
    for c in range(N_CORES):
        m = dict(common)
        m["xf"] = xf[c].astype(bf)
        m["xe"] = xe[c].astype(bf)
        maps.append(m)
    return maps


def kernel(xf, xe, w_kf, b_kf, w_ke, b_ke,
           w_rf, b_rf, g_rf, be_rf, w_re, b_re, g_re, be_re,
           w_bf, b_bf, g_bf, be_bf, w_be, b_be, g_be, be_be):
    # note: conv biases feeding a train-mode BatchNorm cancel exactly
    # (BN subtracts the batch mean), so b_rf/b_re/b_bf/b_be are unused.
    try:
        import jax
        jax.config.update("jax_compilation_cache_dir", "/tmp/jaxcache_kernel")
        jax.config.update("jax_persistent_cache_min_entry_size_bytes", 0)
        jax.config.update("jax_persistent_cache_min_compile_time_secs", 0)
    except Exception:
        pass
    if "nc" not in _CACHE:
        _CACHE["nc"] = build()
    nc = _CACHE["nc"]
    maps = _prep_maps(xf, xe, w_kf, b_kf, w_ke, b_ke, w_rf, g_rf, be_rf,
                      w_re, g_re, be_re, w_bf, g_bf, be_bf, w_be, g_be, be_be)
    res = run_bass_kernel_spmd(nc, maps, core_ids=list(range(N_CORES)))
    gf = np.concatenate(
        [np.asarray(r["gf"]).astype(np.float32).reshape(BL, C, H, W)
         for r in res.results])
    ge = np.concatenate(
        [np.asarray(r["ge"]).astype(np.float32).reshape(BL, C, H, W)
         for r in res.results])
    return gf, ge
